# revision 1
# baseline (speedup 1.0000x reference)
"""DSAM (dual spatial/channel attention) Bass kernel for Trainium2, 8 cores.

Sharding: core c handles batch b=c//4, query-row quarter qi=c%4
(1024 of the 4096 spatial positions). Spatial attention is fused
flash-style (scores -> exp -> weighted sum of V, normalization folded in
via an appended ones-row of V), so the [HW,HW] affinity never touches HBM.
The channel branch (full-image 3x3 conv + 64x64 gram) is computed
redundantly per core.

All inputs are packed into one [65, NIN] array so the whole load is a
single DMA (one semaphore) - PE weight-load instructions only tolerate a
single sync wait, so every matmul operand must trace to one producer
semaphore.

Hardcoded shapes: B=2, C=64, H=W=64, Cq=8.
"""

import numpy as np

EPS = 1e-5
B, C, H, W = 2, 64, 64, 64
HW = H * W
Cq = C // 8
NPAD = 66 * 66 + 2        # per-channel padded flat length (+2 sentinels)
XS_LEN = 18 * 66 + 2      # 18 padded rows slab (+2 sentinels)
NQ = 1024                 # query positions per core

# offsets into the packed input
O_XPA = 0
O_XS = O_XPA + NPAD
O_WS = O_XS + XS_LEN
O_WC = O_WS + 576
O_WQ = O_WC + 576
O_WK = O_WQ + Cq
O_WV = O_WK + Cq
O_WO = O_WV + 65
O_ID = O_WO + 64
O_OB = O_ID + 64
O_CG = O_OB + 1
NIN = O_CG + 1

_CACHE = {}


def _build():
    import concourse.bass as bass
    import concourse.tile as tile
    from concourse import mybir
    from contextlib import ExitStack

    fp = mybir.dt.float32
    f16 = mybir.dt.float16
    AX = mybir.AxisListType.X
    ALU = mybir.AluOpType
    ACTF = mybir.ActivationFunctionType

    nc = bass.Bass()
    in_d = nc.dram_tensor("allin", [65, NIN], fp, kind="ExternalInput")
    out_d = nc.dram_tensor("out", [64, NQ], fp, kind="ExternalOutput")

    with tile.TileContext(nc) as tc, ExitStack() as ctx:
        const = ctx.enter_context(tc.tile_pool(name="const", bufs=1))
        big = ctx.enter_context(tc.tile_pool(name="big", bufs=1))
        work = ctx.enter_context(tc.tile_pool(name="work", bufs=3))
        ps_s = ctx.enter_context(tc.tile_pool(name="ps_s", bufs=2, space="PSUM"))
        ps_u = ctx.enter_context(tc.tile_pool(name="ps_u", bufs=2, space="PSUM"))
        ps_m = ctx.enter_context(tc.tile_pool(name="ps_m", bufs=2, space="PSUM"))

        def fenced(pool, shape, tag):
            # record-keeping no-op wrapper; the wait-strip post-pass below
            # handles PSUM-slot-reuse wait overflow
            return pool.tile(shape, fp, tag=tag, name=tag), []

        allin = big.tile([65, NIN], fp)
        nc.gpsimd.dma_start(allin, in_d[:, :])

        xpa = allin[:, O_XPA:O_XPA + NPAD]
        xs = allin[:, O_XS:O_XS + XS_LEN]
        ws = allin[:, O_WS:O_WS + 576].rearrange("c (t o) -> c t o", t=9)
        wc = allin[:, O_WC:O_WC + 576].rearrange("c (t o) -> c t o", t=9)

        # valid-position views: flat index of pixel (r, w) is (r+1)*66+(w+1)+1
        xpa_v = xpa[:, 68:68 + 64 * 66].rearrange("c (r w) -> c r w", w=66)[:, :, :64]
        xs_v = xs[:, 68:68 + 16 * 66].rearrange("c (r w) -> c r w", w=66)[:, :, :64]

        # DVE-produced copies: dense valid pixels + small weights, so matmuls
        # whose other operand is DVE-produced see a single semaphore.
        xdense = big.tile([65, HW], f16)
        nc.scalar.copy(xdense.rearrange("c (r w) -> c r w", w=64), xpa_v)
        xsdense = big.tile([65, NQ], f16)
        nc.scalar.copy(xsdense.rearrange("c (r w) -> c r w", w=64), xs_v)
        wq = const.tile([65, Cq], f16)
        nc.scalar.copy(wq, allin[:, O_WQ:O_WQ + Cq])
        wk = const.tile([65, Cq], f16)
        nc.scalar.copy(wk, allin[:, O_WK:O_WK + Cq])
        wv = const.tile([65, 65], f16)
        nc.scalar.copy(wv, allin[:, O_WV:O_WV + 65])
        wo = const.tile([64, 64], fp)
        nc.vector.tensor_copy(wo, allin[:64, O_WO:O_WO + 64])
        ident = const.tile([64, 64], fp)
        nc.vector.tensor_copy(ident, allin[:64, O_ID:O_ID + 64])
        ob = const.tile([64, 1], fp)
        nc.vector.tensor_copy(ob, allin[:64, O_OB:O_OB + 1])
        cg = const.tile([64, 1], fp)
        nc.vector.tensor_copy(cg, allin[:64, O_CG:O_CG + 1])

        xpa16 = big.tile([65, NPAD], f16)
        nc.vector.tensor_copy(xpa16, xpa)
        xs16 = big.tile([65, XS_LEN], f16)
        nc.vector.tensor_copy(xs16, xs)
        ws16 = const.tile([65, 9, 64], f16)
        nc.vector.tensor_copy(ws16, ws)
        wc16 = const.tile([65, 9, 64], f16)
        nc.vector.tensor_copy(wc16, wc)

        ones16 = const.tile([1, 64], f16)
        nc.vector.tensor_copy(ones16, xpa_v[64:65, 0, :])

        # persistent SBUF tensors
        k_sb = big.tile([Cq, HW], f16)
        q_sb = big.tile([Cq, NQ], f16)
        vT = big.tile([128, 32, 65], f16)
        cxf = big.tile([64, HW], fp)       # full-image channel-conv fmap (dense)
        fT = big.tile([128, 32, 64], f16)   # fmap transposed chunks
        sxq = big.tile([64, NQ], fp)       # spatial-conv output, our rows
        cxq = big.tile([64, NQ], fp)       # channel-conv output, our rows
        fuse = big.tile([64, NQ], fp)
        out_sb = big.tile([64, NQ + 4], fp)

        # ---------- k / q (1x1 convs; ACT evictions so S_T sees one sem) ----
        for blk in range(8):
            ps, rec = fenced(ps_s, [128, 1024], "S")
            rec.append((nc.tensor.matmul(ps[:Cq, :512], wk,
                        xdense[:, blk * 512:(blk + 1) * 512],
                        start=True, stop=True), 'PE'))
            rec.append((nc.scalar.copy(k_sb[:, blk * 512:(blk + 1) * 512],
                                       ps[:Cq, :512]), 'ACT'))
        for blk in range(2):
            ps, rec = fenced(ps_s, [128, 1024], "S")
            rec.append((nc.tensor.matmul(ps[:Cq, :512], wq,
                        xsdense[:, blk * 512:(blk + 1) * 512],
                        start=True, stop=True), 'PE'))
            rec.append((nc.scalar.copy(q_sb[:, blk * 512:(blk + 1) * 512],
                                       ps[:Cq, :512]), 'ACT'))

        # ---------- vT[j, c] = (x . Wv)[j, c], col 64 = ones ----------
        for grp in range(5):
            n_t = min(7, 32 - grp * 7)
            ps, rec = fenced(ps_s, [128, 1024], "S")
            for t in range(n_t):
                jo = grp * 7 + t
                rec.append((nc.tensor.matmul(ps[:, t * 65:(t + 1) * 65],
                            xdense[:, jo * 128:(jo + 1) * 128], wv,
                            start=True, stop=True), 'PE'))
            rec.append((nc.scalar.copy(vT[:, grp * 7:grp * 7 + n_t, :],
                                       ps[:, :n_t * 65]), 'ACT'))

        # ---------- full-image channel conv (padded-space accumulation) -----
        rows_done = 0
        while rows_done < 64:
            rows = min(7, 64 - rows_done)
            fsz = rows * 66
            p0 = rows_done * 66
            ps, rec = fenced(ps_m, [128, 512], "m")
            for tap in range(9):
                dy, dx = tap // 3, tap % 3
                off = dy * 66 + dx
                rec.append((nc.tensor.matmul(ps[:64, :fsz], wc16[:, tap, :],
                            xpa16[:, off + p0: off + p0 + fsz],
                            start=(tap == 0), stop=(tap == 8)), 'PE'))
            pv = ps[:64, :fsz].rearrange("c (r w) -> c r w", w=66)[:, :, 1:65]
            rec.append((nc.vector.tensor_scalar_max(
                cxf[:, rows_done * 64:(rows_done + rows) * 64], pv, 0.0), 'DVE'))
            rows_done += rows

        # ---------- fmap transpose chunks ----------
        for grp in range(4):
            ps, rec = fenced(ps_m, [128, 512], "m")
            for t in range(8):
                jo = grp * 8 + t
                rec.append((nc.tensor.transpose(ps[:, t * 64:(t + 1) * 64],
                            cxf[:, jo * 128:(jo + 1) * 128], ident), 'PE'))
            rec.append((nc.vector.tensor_copy(fT[:, grp * 8:(grp + 1) * 8, :],
                                              ps), 'DVE'))

        # ---------- our-rows convs (spatial WS -> sxq, channel WC -> cxq) ---
        for wmat, dst in ((ws16, sxq), (wc16, cxq)):
            for bi, rows in enumerate((7, 7, 2)):
                fsz = rows * 66
                p0 = (0, 462, 924)[bi]
                ps, rec = fenced(ps_m, [128, 512], "m")
                for tap in range(9):
                    dy, dx = tap // 3, tap % 3
                    off = dy * 66 + dx
                    rec.append((nc.tensor.matmul(ps[:64, :fsz], wmat[:, tap, :],
                                xs16[:, off + p0: off + p0 + fsz],
                                start=(tap == 0), stop=(tap == 8)), 'PE'))
                pv = ps[:64, :fsz].rearrange("c (r w) -> c r w", w=66)[:, :, 1:65]
                rec.append((nc.vector.tensor_scalar_max(
                    dst[:, p0 // 66 * 64:(p0 // 66 + rows) * 64], pv, 0.0), 'DVE'))

        # ---------- spatial attention (flash-style) ----------
        for ib in range(2):
            psU, recU = fenced(ps_u, [65, 512], "U")
            for rnd in range(16):
                psS, rec = fenced(ps_s, [128, 1024], "S")
                for hh in range(2):
                    jo = rnd * 2 + hh
                    rec.append((nc.tensor.matmul(
                        psS[:, hh * 512:(hh + 1) * 512],
                        k_sb[:, jo * 128:(jo + 1) * 128],
                        q_sb[:, ib * 512:(ib + 1) * 512],
                        start=True, stop=True), 'PE'))
                Et = work.tile([128, 1024], f16, tag="E")
                et_last = Et
                rec.append((nc.scalar.activation(Et, psS, ACTF.Exp), 'ACT'))
                for hh in range(2):
                    jo = rnd * 2 + hh
                    recU.append((nc.tensor.matmul(psU, vT[:, jo, :],
                                 Et[:, hh * 512:(hh + 1) * 512],
                                 start=(jo == 0), stop=(jo == 31)), 'PE'))
            rcp = work.tile([1, 512], f16, tag="rec")
            with nc.allow_low_precision(reason="1/denom broadcast via f16 matmul"):
                nc.vector.reciprocal(rcp, psU[64:65, :])
            U_sb = work.tile([64, 512], fp, tag="U_sb")
            nc.vector.tensor_copy(U_sb, psU[:64, :])
            # broadcast 1/denom across partitions via a K=1 matmul with ones
            psB, recB = fenced(ps_m, [128, 512], "m")
            recB.append((nc.tensor.matmul(psB[:64, :], ones16, rcp,
                                          start=True, stop=True), 'PE'))
            rec64 = work.tile([64, 512], fp, tag="rec64")
            recB.append((nc.vector.tensor_copy(rec64, psB[:64, :]), 'DVE'))
            fb = fuse[:, ib * 512:(ib + 1) * 512]
            nc.vector.tensor_mul(fb, U_sb, rec64)
            nc.vector.tensor_add(fb, fb, sxq[:, ib * 512:(ib + 1) * 512])

        # ---------- channel attention ----------
        psA_t, recA = fenced(ps_m, [128, 512], "m")
        psA = psA_t[:64, :64]
        for jo in range(32):
            recA.append((nc.tensor.matmul(psA, fT[:, jo, :], fT[:, jo, :],
                         start=(jo == 0), stop=(jo == 31)), 'PE'))
        Ac = work.tile([64, 64], fp, tag="ac_sb")
        recA.append((nc.vector.tensor_copy(Ac, psA), 'DVE'))
        mn = work.tile([64, 1], fp, tag="mn")
        nc.vector.tensor_reduce(mn, Ac, AX, ALU.min)
        Ec = work.tile([64, 64], fp, tag="ec")
        # exp(mn - Ac): softmax(max-Ac) == softmax(-Ac), stabilized by row min
        nc.scalar.activation(Ec, Ac, ACTF.Exp, bias=mn, scale=-1.0)
        sm = work.tile([64, 1], fp, tag="sm")
        nc.vector.reduce_sum(sm, Ec, AX)
        rc = work.tile([64, 1], fp, tag="rc")
        nc.vector.reciprocal(rc, sm)
        # Ec := Ec * (1/sum) * c_gamma
        nc.vector.tensor_scalar(Ec, Ec, rc, cg, ALU.mult, ALU.mult)
        psT_t, recT = fenced(ps_m, [128, 512], "m")
        psT = psT_t[:64, :64]
        recT.append((nc.tensor.transpose(psT, Ec, ident), 'PE'))
        ScT = work.tile([64, 64], fp, tag="sct_sb")
        recT.append((nc.vector.tensor_copy(ScT, psT), 'DVE'))
        for ib in range(2):
            psC_t, recC = fenced(ps_m, [128, 512], "m")
            psC = psC_t[:64, :]
            recC.append((nc.tensor.matmul(psC, ScT,
                         cxq[:, ib * 512:(ib + 1) * 512],
                         start=True, stop=True), 'PE'))
            fb = fuse[:, ib * 512:(ib + 1) * 512]
            recC.append((nc.vector.tensor_add(fb, fb, psC), 'DVE'))
            nc.vector.tensor_add(fb, fb, cxq[:, ib * 512:(ib + 1) * 512])

        # ---------- output 1x1 conv ----------
        for ib in range(2):
            psO_t, recO = fenced(ps_m, [128, 512], "m")
            psO = psO_t[:64, :]
            recO.append((nc.tensor.matmul(psO, wo,
                         fuse[:, ib * 512:(ib + 1) * 512],
                         start=True, stop=True), 'PE'))
            recO.append((nc.vector.tensor_scalar_add(
                out_sb[:, ib * 512:(ib + 1) * 512], psO, ob), 'DVE'))
        # funnel ACT's tail into out_sb so the output DMA transitively
        # covers every engine; the final drain then only waits on the DMA
        nc.vector.tensor_copy(out_sb[0:1, NQ:NQ + 4], et_last[0:1, 0:4])
        nc.gpsimd.dma_start(out_d[:, :], out_sb[:, :NQ])

    # Engine instructions encode at most one sync wait. Where Tile emitted
    # two, one is always the own-engine wait for a slot WAW/WAR; engine
    # queues are FIFO and the kept cross-engine wait transitively covers the
    # own-engine one (the slot's reader waited on those writers). Strip it.
    eng_sem = {'PE': 'PE', 'Activation': 'Activation', 'DVE': 'DVE',
               'Pool': 'Pool', 'SP': 'SP'}
    for blk in nc.m.functions[0].blocks:
        for ins in blk.instructions:
            si = ins.sync_info
            if si is None or len(si.on_wait) <= 1:
                continue
            eng = str(getattr(ins, 'engine', '')).replace('EngineType.', '')
            if eng not in ('PE', 'Activation', 'DVE', 'Pool'):
                continue  # SP/sync instructions allow many waits
            own = eng_sem.get(eng, eng)
            keep = [w for w in si.on_wait if not w.ant_name.startswith(own)]
            assert len(keep) == 1, \
                (ins.name, eng, [w.ant_name for w in si.on_wait])
            si.on_wait = keep
            ins.sync_info = si

    # Tail drains join every engine, but their wait budget is 1. The output
    # DMA transitively covers every engine (it reads out_sb, whose writers
    # funnel ACT/PE/DVE), so the drain only needs the out-DMA queue's sem.
    last_dma_sem = None
    for blk in nc.m.functions[0].blocks:
        for ins in blk.instructions:
            si = ins.sync_info
            if si is None:
                continue
            for u in si.on_update:
                if u.ant_name.startswith('DMA'):
                    last_dma_sem = u.ant_name
    for blk in nc.m.functions[0].blocks:
        for ins in blk.instructions:
            si = ins.sync_info
            if si is None or type(ins).__name__ != 'InstDrain':
                continue
            if len(si.on_wait) > 1:
                keep = [w for w in si.on_wait if w.ant_name == last_dma_sem]
                if keep:
                    si.on_wait = keep
                    ins.sync_info = si

    return nc


def _prep_host(inputs):
    x = np.asarray(inputs['x'], np.float32)

    def fold(Wc, bc, g, b_, m, v):
        sc = np.asarray(g) / np.sqrt(np.asarray(v) + EPS)
        return (np.asarray(Wc) * sc[:, None, None, None],
                (np.asarray(bc) - np.asarray(m)) * sc + np.asarray(b_))

    sWf, sbf = fold(inputs['sW'], inputs['sb'], inputs['s_g'], inputs['s_b'],
                    inputs['s_m'], inputs['s_v'])
    cWf, cbf = fold(inputs['cW'], inputs['cb'], inputs['c_g'], inputs['c_b'],
                    inputs['c_m'], inputs['c_v'])

    def taps(Wf, bf_):
        out = np.zeros((65, 9, 64), np.float32)
        for dy in range(3):
            for dx in range(3):
                out[:64, dy * 3 + dx, :] = Wf[:, :, dy, dx].T
        out[64, 4, :] = bf_
        return out.reshape(65, 9 * 64)

    base = np.zeros((65, NIN), np.float32)
    base[:, O_WS:O_WS + 576] = taps(sWf, sbf)
    base[:, O_WC:O_WC + 576] = taps(cWf, cbf)
    base[:64, O_WQ:O_WQ + Cq] = np.asarray(inputs['qW'])[:, :, 0, 0].T
    base[64, O_WQ:O_WQ + Cq] = np.asarray(inputs['qb'])
    base[:64, O_WK:O_WK + Cq] = np.asarray(inputs['kW'])[:, :, 0, 0].T
    base[64, O_WK:O_WK + Cq] = np.asarray(inputs['kb'])
    sg = float(np.asarray(inputs['s_gamma'])[0])
    base[:64, O_WV:O_WV + 64] = np.asarray(inputs['vW'])[:, :, 0, 0].T * sg
    base[64, O_WV:O_WV + 64] = np.asarray(inputs['vb']) * sg
    base[64, O_WV + 64] = 1.0
    base[:64, O_WO:O_WO + 64] = np.asarray(inputs['oW'])[:, :, 0, 0].T
    base[:64, O_ID:O_ID + 64] = np.eye(64, dtype=np.float32)
    base[:64, O_OB] = np.asarray(inputs['ob'])
    base[:64, O_CG] = float(np.asarray(inputs['c_gamma'])[0])

    in_maps = []
    for c in range(8):
        b, qi = c // 4, c % 4
        m = base.copy()
        xp = np.zeros((65, 66, 66), np.float32)
        xp[:64, 1:65, 1:65] = x[b]
        xp[64, 1:65, 1:65] = 1.0
        m[:, O_XPA + 1:O_XPA + 1 + 66 * 66] = xp.reshape(65, 66 * 66)
        m[:, O_XS + 1:O_XS + 1 + 18 * 66] = \
            xp[:, qi * 16:qi * 16 + 18, :].reshape(65, 18 * 66)
        in_maps.append({'allin': np.ascontiguousarray(m)})
    return in_maps


def kernel(**inputs):
    from concourse.bass_utils import run_bass_kernel_spmd
    if 'nc' not in _CACHE:
        _CACHE['nc'] = _build()
    nc = _CACHE['nc']
    in_maps = _prep_host(inputs)
    res = run_bass_kernel_spmd(nc, in_maps, core_ids=list(range(8)))
    out = np.zeros((B, C, H, W), np.float32)
    for c in range(8):
        b, qi = c // 4, c % 4
        out[b, :, qi * 16:(qi + 1) * 16, :] = \
            res.results[c]['out'].reshape(64, 16, 64)
    return out



# revision 28
# speedup vs baseline: 1.4750x; 1.4750x over previous
"""DSAM (dual spatial/channel attention) Bass kernel for Trainium2, 8 cores.

Sharding: core c handles batch b=c//4, query-row quarter qi=c%4
(1024 of the 4096 spatial positions). qi enters only through per-core
host-packed input regions (a dense q-block and a padded 18-row slab for
the our-rows convs), so one compiled program serves all cores. Spatial
attention is fused flash-style (scores -> exp -> weighted sum of V,
normalization via an appended ones-column of V). The channel branch
(full-image 3x3 conv + 64x64 gram) is computed redundantly per core.

v2 optimizations over the v1 baseline (95.5us):
- All inputs host-packed f16 into one dram tensor, DMA'd as three slices
  on one queue (one semaphore, rising thresholds) so every matmul
  operand still traces to a single-sem wait.
- 3x3 convs use tap pairing: x is stored twice (second copy shifted one
  pixel) so taps (dy,0)+(dy,1) contract K=128 in one matmul; 6 matmuls
  per block instead of 9. Bias+relu fold into the psum eviction
  (tensor_scalar add+max), so no ones-row is needed for conv biases.
- k is produced partition-packed ([128,1024]: row 32r+d = k_d for the
  r-th quarter of j, via tile_position col-groups) and q replicated at
  4 row offsets via a [65,128] weight; evictions are two 1024-free ACT
  copies instead of ten 512-free ones. Scores matmuls then run as
  row-group (32r, 0) tiles with both operands at base partition 32r.
- The exp@V accumulation runs in fp8e4 DoubleRow: vT stored as fp8
  pairs [128, 2, 80-stride], Et written by ACT directly as fp8, one
  matmul per 256 contracted j at 0.5 cycles/col.
- Conv/transpose/gram/softmax work is interleaved into the attention
  rounds as "fillers" to run under the ACT-bound exp stream.

Hardcoded shapes: B=2, C=64, H=W=64, Cq=8.
"""

import numpy as np

EPS = 1e-5
B, C, H, W = 2, 64, 64, 64
HW = H * W
Cq = C // 8
NQ = 1024                 # query positions per core

# ---- packed input layout (one f16 dram tensor [128, NIN]) ----
O_WSP = 0                 # [128, 3, 64] spatial conv pair taps
O_WCP = O_WSP + 192       # [128, 3, 64] channel conv pair taps
O_WSS = O_WCP + 192       # [64, 3, 64] spatial solo taps (dx=2)
O_WCS = O_WSS + 192       # [64, 3, 64] channel solo taps
O_WQR = O_WCS + 192       # [65, 128] q weights replicated at 4 offsets
O_WK = O_WQR + 128        # [65, 32] (cols 8..32 zero: initialize full rows)
O_WV = O_WK + 32          # [65, 65] (x s_gamma, ones col 64, bias row 64)
O_WO = O_WV + 65          # [64, 64]
O_ID = O_WO + 64          # [64, 64] f16 identity
O_SB = O_ID + 64          # [64, 1] spatial conv bias (bn-folded)
O_CB = O_SB + 1           # [64, 1] channel conv bias
O_OB = O_CB + 1           # [64, 1] out conv bias
O_CG = O_OB + 1           # [64, 1] c_gamma
O_ONE = O_CG + 1          # [1, 64] ones at partition 0 (bcast lhsT)
O_W_END = O_ONE + 64
O_XQ = 1280               # [65, 1024] dense x, our query block + ones row
O_X16 = O_XQ + 1024       # [65, 4096] dense x + ones row (row 64)
O_X2 = O_X16 + 4096       # [128, 4360] padded x; rows 64.. shifted by 1
O_XS2 = O_X2 + 4360       # [128, 1196] padded 18-row slab, stacked-shifted
NIN = O_XS2 + 1196

_CACHE = {}


def _build(use_fp8=True):
    import concourse.bass as bass
    import concourse.tile as tile
    from concourse import mybir
    from contextlib import ExitStack

    fp = mybir.dt.float32
    f16 = mybir.dt.float16
    f8 = mybir.dt.float8e4
    AX = mybir.AxisListType.X
    ALU = mybir.AluOpType
    ACTF = mybir.ActivationFunctionType
    DR = mybir.MatmulPerfMode.DoubleRow
    e_dt = f8 if use_fp8 else f16

    nc = bass.Bass()
    in_d = nc.dram_tensor("allin", [128, NIN], f16, kind="ExternalInput")
    out_d = nc.dram_tensor("out", [64, NQ], fp, kind="ExternalOutput")

    with tile.TileContext(nc) as tc, ExitStack() as ctx:
        const = ctx.enter_context(tc.tile_pool(name="const", bufs=1))
        big = ctx.enter_context(tc.tile_pool(name="big", bufs=1))
        work = ctx.enter_context(tc.tile_pool(name="work", bufs=3))
        ps_s = ctx.enter_context(tc.tile_pool(name="ps_s", bufs=2, space="PSUM"))
        ps_u = ctx.enter_context(tc.tile_pool(name="ps_u", bufs=2, space="PSUM"))
        ps_m = ctx.enter_context(tc.tile_pool(name="ps_m", bufs=2, space="PSUM"))

        allin = big.tile([128, NIN], f16)
        # DMA slices on one queue: same semaphore, rising thresholds
        nc.gpsimd.dma_start(allin[:, :O_W_END], in_d[:, :O_W_END])
        nc.gpsimd.dma_start(allin[:65, O_XQ:O_X16], in_d[:65, O_XQ:O_X16])
        nc.gpsimd.dma_start(allin[:65, O_X16:O_X16 + 4096],
                            in_d[:65, O_X16:O_X16 + 4096])
        nc.gpsimd.dma_start(allin[:, O_XS2:], in_d[:, O_XS2:])
        nc.gpsimd.dma_start(allin[:, O_X2:O_XS2], in_d[:, O_X2:O_XS2])

        wsp = allin[:, O_WSP:O_WSP + 192].rearrange("c (t o) -> c t o", t=3)
        wcp = allin[:, O_WCP:O_WCP + 192].rearrange("c (t o) -> c t o", t=3)
        wss = allin[:64, O_WSS:O_WSS + 192].rearrange("c (t o) -> c t o", t=3)
        wcs = allin[:64, O_WCS:O_WCS + 192].rearrange("c (t o) -> c t o", t=3)
        wqr = allin[:65, O_WQR:O_WQR + 128]
        wk = allin[:65, O_WK:O_WK + 32]
        wv = allin[:65, O_WV:O_WV + 65]
        wo = allin[:64, O_WO:O_WO + 64]
        ident = allin[:64, O_ID:O_ID + 64]
        xq = allin[:65, O_XQ:O_XQ + 1024]
        x16 = allin[:65, O_X16:O_X16 + 4096]
        ones64 = allin[0:1, O_ONE:O_ONE + 64]     # lhsT for 1->64 broadcast
        x2 = allin[:, O_X2:O_X2 + 4360]
        x2s = allin[:64, O_X2:O_X2 + 4360]        # top half (solo taps)
        xs2 = allin[:, O_XS2:O_XS2 + 1196]
        xs2s = allin[:64, O_XS2:O_XS2 + 1196]

        # persistent SBUF tensors
        k2 = big.tile([128, 1024], f16)     # row 32r+d = k_d, j = 1024r + f
        qrep = big.tile([128, 1024], f16)   # row 32r+d = q_d (replicated)
        vT8 = big.tile([128, 32, 80], e_dt)  # [p, jo, c]: v'[c, 128jo+p]
        cxf = big.tile([64, HW], f16)       # full-image channel-conv fmap
        fT = big.tile([128, 32, 64], f16)   # fmap transposed chunks
        sxq = big.tile([64, NQ], f16)       # spatial-conv output, our rows
        cxq = big.tile([64, NQ], f16)       # channel-conv output, our rows
        fuse = big.tile([64, NQ], f16)
        Ac = const.tile([64, 64], fp)       # gram accumulator (DVE-summed)
        ScT = const.tile([64, 64], f16)
        out_sb = big.tile([64, NQ + 4], fp)

        # f32 copies of the scalar operands (TS scalars must be f32; a DVE
        # copy keeps them single-producer so evictions strip to one wait)
        bias32 = const.tile([64, 4], fp)
        nc.vector.tensor_copy(bias32, allin[:64, O_SB:O_SB + 4])
        sbias = bias32[:, 0:1]
        cbias = bias32[:, 1:2]
        ob = bias32[:, 2:3]
        cg = bias32[:, 3:4]

        # ---------- k: partition-packed [128, 1024] ----------
        # evictions go through ACT so scores matmuls see a single sem class
        # (qrep operand + psum-slot WAR vs ACT exp both resolve to ACT).
        psK = ps_s.tile([128, 1024], fp, tag="S", name="psK")
        for r in range(4):
            for h in range(2):
                nc.tensor.matmul(
                    psK[32 * r:32 * r + 32, 512 * h:512 * h + 512], wk,
                    x16[:, 1024 * r + 512 * h:1024 * r + 512 * h + 512],
                    start=True, stop=True, tile_position=(0, 32 * r))
        nc.scalar.copy(k2, psK)
        psQ = ps_s.tile([128, 1024], fp, tag="S", name="psQ")
        for h in range(2):
            nc.tensor.matmul(psQ[:, 512 * h:512 * h + 512], wqr,
                             xq[:, 512 * h:512 * h + 512],
                             start=True, stop=True)
        nc.scalar.copy(qrep, psQ)

        # ---------- vT8[p, jo, c] = v'[c, j=128*jo+p], col 64 = ones ------
        for grp in range(5):
            n_t = min(7, 32 - grp * 7)
            ps = ps_m.tile([128, 512], fp, tag="m", name="psv%d" % grp)
            for t in range(n_t):
                jo = grp * 7 + t
                nc.tensor.matmul(ps[:, t * 65:(t + 1) * 65],
                                 x16[:, jo * 128:(jo + 1) * 128], wv,
                                 start=True, stop=True)
            nc.vector.tensor_copy(
                vT8[:, grp * 7:grp * 7 + n_t, :65],
                ps[:, :n_t * 65].rearrange("p (t c) -> p t c", t=n_t))

        # absorb the full input-DMA wait into PE's history with a dummy
        # weight load (next real matmul reloads weights anyway), so later
        # conv matmuls only carry their psum-WAR wait
        nc.tensor.ldweights(x2[0:1, :64])
        nc.tensor.ldweights(xs2[0:1, :64])

        # ---------- filler steps: convs, fT transposes, gram, softmax -----
        fillers = []

        def conv_block(srcp, srcs, wp, ws_, bias_, dst, r0, rows, blkname):
            fsz = rows * 66
            p0 = r0 * 66
            ps = ps_m.tile([128, 512], fp, tag="m", name=blkname)
            pc = ps[:64, :fsz]
            steps = []
            for i, dy in enumerate((0, 1, 2)):
                off = dy * 66 + p0
                steps.append((wp[:, dy, :], srcp[:, off:off + fsz],
                              i == 0, False))
            for i, dy in enumerate((0, 1, 2)):
                off = dy * 66 + 2 + p0
                steps.append((ws_[:, dy, :], srcs[:, off:off + fsz],
                              False, i == 2))
            out = []
            for w_, rhs, st, sp in steps:
                def mk(w__=w_, rhs__=rhs, st__=st, sp__=sp):
                    nc.tensor.matmul(pc, w__, rhs__, start=st__, stop=sp__)
                out.append(mk)

            def evict():
                pv = pc.rearrange("c (r w) -> c r w", w=66)[:, :, 0:64]
                dv = dst[:, r0 * 64:(r0 + rows) * 64] \
                    .rearrange("c (r w) -> c r w", w=64)
                nc.vector.tensor_scalar(dv, pv, bias_, 0.0, ALU.add, ALU.max)
            out[-1] = (out[-1], evict)
            return out

        # full-image channel conv: 9 blocks of 7 rows + 1 of 1 row
        rows_done, bi = 0, 0
        while rows_done < 64:
            rows = min(7, 64 - rows_done)
            fillers.extend(conv_block(x2, x2s, wcp, wcs, cbias, cxf,
                                      rows_done, rows, "cc%d" % bi))
            rows_done += rows
            bi += 1
        # our-rows convs from the 18-row slab (slab row r = query row r)
        for bi2, (r0, rows) in enumerate(((0, 7), (7, 7), (14, 2))):
            fillers.extend(conv_block(xs2, xs2s, wsp, wss, sbias, sxq,
                                      r0, rows, "sc%d" % bi2))
            fillers.extend(conv_block(xs2, xs2s, wcp, wcs, cbias, cxq,
                                      r0, rows, "qc%d" % bi2))

        # fT transpose chunks + gram partials (4 groups of 8 chunks)
        for grp in range(4):
            psT = ps_m.tile([128, 1024], f16, tag="m", name="ft%d" % grp)
            steps = []
            for t in range(8):
                jo = grp * 8 + t

                def mk(t_=t, jo_=jo, psT_=psT):
                    nc.tensor.transpose(psT_[:, t_ * 64:(t_ + 1) * 64],
                                        cxf[:, jo_ * 128:(jo_ + 1) * 128],
                                        ident)
                steps.append(mk)

            def evict_ft(g=grp, psT_=psT):
                nc.vector.tensor_copy(
                    fT[:, g * 8:(g + 1) * 8, :],
                    psT_[:, :512].rearrange("p (t c) -> p t c", t=8))
            steps[-1] = (steps[-1], evict_ft)
            fillers.extend(steps)
            psA = ps_m.tile([128, 512], fp, tag="m", name="gr%d" % grp)
            gsteps = []
            for t in range(8):
                jo = grp * 8 + t

                def mkg(t_=t, jo_=jo, psA_=psA):
                    nc.tensor.matmul(psA_[:64, :64], fT[:, jo_, :],
                                     fT[:, jo_, :],
                                     start=(t_ == 0), stop=(t_ == 7))
                gsteps.append(mkg)

            def evict_gr(g=grp, psA_=psA):
                if g == 0:
                    nc.vector.tensor_copy(Ac, psA_[:64, :64])
                else:
                    nc.vector.tensor_add(Ac, Ac, psA_[:64, :64])
            gsteps[-1] = (gsteps[-1], evict_gr)
            fillers.extend(gsteps)

        def chan_softmax():
            # softmax(max-Ac) == softmax(-Ac), stabilized by row min
            mn = work.tile([64, 1], fp, tag="mn")
            nc.vector.tensor_reduce(mn, Ac, AX, ALU.min)
            Ec = work.tile([64, 64], f16, tag="ec")
            nc.scalar.activation(Ec, Ac, ACTF.Exp, bias=mn, scale=-1.0)
            sm = work.tile([64, 1], fp, tag="sm")
            nc.vector.reduce_sum(sm, Ec, AX)
            rc = work.tile([64, 1], fp, tag="rc")
            nc.vector.reciprocal(rc, sm)
            nc.vector.tensor_scalar(Ec, Ec, rc, cg, ALU.mult, ALU.mult)
            psT = ps_m.tile([128, 1024], f16, tag="m", name="sct")
            nc.tensor.transpose(psT[:64, :64], Ec, ident)
            nc.vector.tensor_copy(ScT, psT[:64, :64])
        fillers.append(chan_softmax)

        # ---------- spatial attention with interleaved fillers ----------
        fill_i = [0]

        def pop_fillers(n):
            k = 0
            while k < n and fill_i[0] < len(fillers):
                f = fillers[fill_i[0]]
                fill_i[0] += 1
                if isinstance(f, tuple):
                    for g in f:
                        g()
                else:
                    f()
                k += 1

        U_sb = [None, None]
        rcp = [None, None]
        et_last = [None]
        for ib in range(2):
            psU = ps_u.tile([65, 512], fp, tag="U", name="psU%d" % ib)
            for rnd in range(16):
                psS = ps_s.tile([128, 1024], fp, tag="S",
                                name="psS%d_%d" % (ib, rnd))
                for hh in range(2):
                    jo = rnd * 2 + hh
                    r, jj = jo // 8, jo % 8
                    nc.tensor.matmul(
                        psS[:, hh * 512:(hh + 1) * 512],
                        k2[32 * r:32 * r + Cq, jj * 128:(jj + 1) * 128],
                        qrep[32 * r:32 * r + Cq, ib * 512:(ib + 1) * 512],
                        start=True, stop=True,
                        tile_position=(32 * r, 0))
                Et = work.tile([128, 1024], e_dt, tag="E")
                et_last[0] = Et
                nc.scalar.activation(Et, psS, ACTF.Exp)
                if use_fp8:
                    nc.tensor.matmul(
                        psU, vT8[:, 2 * rnd:2 * rnd + 2, :65],
                        Et.rearrange("p (t i) -> p t i", t=2),
                        start=(rnd == 0), stop=(rnd == 15), perf_mode=DR)
                else:
                    for hh in range(2):
                        jo = rnd * 2 + hh
                        nc.tensor.matmul(psU, vT8[:, jo, :65],
                                         Et[:, hh * 512:(hh + 1) * 512],
                                         start=(jo == 0), stop=(jo == 31))
                pop_fillers(3)
            # rcp = 1/denominator (ones-row of v' at column 64)
            rcp_t = work.tile([1, 512], f16, tag="rcp%d" % ib)
            with nc.allow_low_precision(reason="1/denom bcast via f16 mm"):
                nc.vector.reciprocal(rcp_t, psU[64:65, :])
            U_t = work.tile([64, 512], f16, tag="U_sb%d" % ib)
            nc.vector.tensor_copy(U_t, psU[:64, :])
            U_sb[ib] = U_t
            rcp[ib] = rcp_t
        pop_fillers(len(fillers))

        # ---------- fuse + output conv ----------
        for ib in range(2):
            sl = slice(ib * 512, (ib + 1) * 512)
            psB = ps_m.tile([128, 512], fp, tag="m", name="bc%d" % ib)
            nc.tensor.matmul(psB[:64, :], ones64, rcp[ib],
                             start=True, stop=True)
            fb = fuse[:, sl]
            nc.vector.tensor_mul(fb, U_sb[ib], psB[:64, :])
            nc.vector.tensor_add(fb, fb, sxq[:, sl])
            nc.vector.tensor_add(fb, fb, cxq[:, sl])
            psC = ps_m.tile([128, 512], fp, tag="m", name="ca%d" % ib)
            nc.tensor.matmul(psC[:64, :], ScT, cxq[:, sl],
                             start=True, stop=True)
            nc.vector.tensor_add(fb, fb, psC[:64, :])
            psO = ps_m.tile([128, 512], fp, tag="m", name="oc%d" % ib)
            nc.tensor.matmul(psO[:64, :], wo, fb, start=True, stop=True)
            nc.vector.tensor_scalar_add(out_sb[:, sl], psO[:64, :], ob)

        # funnel ACT's tail into out_sb so the output DMA transitively
        # covers every engine; the final drain then only waits on the DMA
        nc.vector.tensor_copy(out_sb[0:1, NQ:NQ + 4], et_last[0][0:1, 0:4])
        nc.gpsimd.dma_start(out_d[:, :], out_sb[:, :NQ])

    _strip_waits(nc)
    return nc


def _wait_thr(w):
    for a in ('wait_value', 'threshold', 'value', 'target'):
        v = getattr(w, a, None)
        if v is not None:
            return v
    return 0


def _strip_waits(nc):
    """Engine instructions encode at most one sync wait.
    1) Merge waits on the same semaphore (keep max threshold).
    2) Where >1 sem remains, drop the own-engine wait: engine queues are
       FIFO and the kept cross-engine wait transitively covers it (the
       slot's reader waited on those writers).
    3) Tail drains only need the out-DMA queue's semaphore (the output
       DMA transitively covers every engine)."""
    eng_sem = {'PE': 'PE', 'Activation': 'Activation', 'DVE': 'DVE',
               'Pool': 'Pool', 'SP': 'SP'}
    # per-engine history of already-waited (sem -> max threshold): engine
    # queues are FIFO, so any wait satisfied before an earlier instruction
    # on the same engine is satisfied for all later ones.
    def strippable(w):
        return (str(w.wait_mode) == 'sem-ge-imm'
                and 'barrier' not in w.ant_name)

    hist = {}
    for blk in nc.m.functions[0].blocks:
        for ins in blk.instructions:
            si = ins.sync_info
            if si is None or not si.on_wait:
                continue
            eng = str(getattr(ins, 'engine', '')).replace('EngineType.', '')
            if eng not in ('PE', 'Activation', 'DVE', 'Pool') or \
                    type(ins).__name__ in ('InstEventSemaphore', 'InstDrain'):
                continue  # sync instructions keep their waits untouched
            h = hist.setdefault(eng, {})
            fixed = [w for w in si.on_wait if not strippable(w)]
            by_sem = {}
            for w in si.on_wait:
                if not strippable(w):
                    continue
                prev = by_sem.get(w.ant_name)
                if prev is None or _wait_thr(w) > _wait_thr(prev):
                    by_sem[w.ant_name] = w
            keep = [w for w in by_sem.values()
                    if _wait_thr(w) > h.get(w.ant_name, -1)]
            if len(keep) > 1:
                own = eng_sem.get(eng, eng)
                cross = [w for w in keep if not w.ant_name.startswith(own)]
                if cross:
                    keep = cross
            assert len(keep) + len(fixed) <= 1, \
                (ins.name, eng,
                 [(w.ant_name, _wait_thr(w)) for w in si.on_wait])
            for w in by_sem.values():
                h[w.ant_name] = max(h.get(w.ant_name, 0), _wait_thr(w))
            si.on_wait = fixed + keep
            ins.sync_info = si

    last_dma_sem = None
    for blk in nc.m.functions[0].blocks:
        for ins in blk.instructions:
            si = ins.sync_info
            if si is None:
                continue
            for u in si.on_update:
                if u.ant_name.startswith('DMA'):
                    last_dma_sem = u.ant_name
    for blk in nc.m.functions[0].blocks:
        for ins in blk.instructions:
            si = ins.sync_info
            if si is None or type(ins).__name__ != 'InstDrain':
                continue
            if len(si.on_wait) > 1:
                keep = [w for w in si.on_wait if w.ant_name == last_dma_sem]
                if keep:
                    si.on_wait = keep
                    ins.sync_info = si


def _prep_host(inputs):
    x = np.asarray(inputs['x'], np.float32)

    def fold(Wc, bc, g, b_, m, v):
        sc = np.asarray(g) / np.sqrt(np.asarray(v) + EPS)
        return (np.asarray(Wc) * sc[:, None, None, None],
                (np.asarray(bc) - np.asarray(m)) * sc + np.asarray(b_))

    sWf, sbf = fold(inputs['sW'], inputs['sb'], inputs['s_g'], inputs['s_b'],
                    inputs['s_m'], inputs['s_v'])
    cWf, cbf = fold(inputs['cW'], inputs['cb'], inputs['c_g'], inputs['c_b'],
                    inputs['c_m'], inputs['c_v'])

    base = np.zeros((128, NIN), np.float16)
    for dy in range(3):
        base[:64, O_WSP + dy * 64:O_WSP + (dy + 1) * 64] = sWf[:, :, dy, 0].T
        base[64:, O_WSP + dy * 64:O_WSP + (dy + 1) * 64] = sWf[:, :, dy, 1].T
        base[:64, O_WCP + dy * 64:O_WCP + (dy + 1) * 64] = cWf[:, :, dy, 0].T
        base[64:, O_WCP + dy * 64:O_WCP + (dy + 1) * 64] = cWf[:, :, dy, 1].T
        base[:64, O_WSS + dy * 64:O_WSS + (dy + 1) * 64] = sWf[:, :, dy, 2].T
        base[:64, O_WCS + dy * 64:O_WCS + (dy + 1) * 64] = cWf[:, :, dy, 2].T
    wq2 = np.asarray(inputs['qW'])[:, :, 0, 0].T  # [C, Cq]
    for r in range(4):
        base[:64, O_WQR + 32 * r:O_WQR + 32 * r + Cq] = wq2
        base[64, O_WQR + 32 * r:O_WQR + 32 * r + Cq] = np.asarray(inputs['qb'])
    base[:64, O_WK:O_WK + Cq] = np.asarray(inputs['kW'])[:, :, 0, 0].T
    base[64, O_WK:O_WK + Cq] = np.asarray(inputs['kb'])
    sg = float(np.asarray(inputs['s_gamma'])[0])
    base[:64, O_WV:O_WV + 64] = np.asarray(inputs['vW'])[:, :, 0, 0].T * sg
    base[64, O_WV:O_WV + 64] = np.asarray(inputs['vb']) * sg
    base[64, O_WV + 64] = 1.0
    base[:64, O_WO:O_WO + 64] = np.asarray(inputs['oW'])[:, :, 0, 0].T
    base[:64, O_ID:O_ID + 64] = np.eye(64, dtype=np.float16)
    base[:64, O_SB] = sbf
    base[:64, O_CB] = cbf
    base[:64, O_OB] = np.asarray(inputs['ob'])
    base[:64, O_CG] = float(np.asarray(inputs['c_gamma'])[0])
    base[0, O_ONE:O_ONE + 64] = 1.0

    in_maps = []
    for c in range(8):
        b, qi = c // 4, c % 4
        m = base.copy()
        xb = x[b]
        m[:64, O_X16:O_X16 + 4096] = xb.reshape(64, 4096)
        m[64, O_X16:O_X16 + 4096] = 1.0
        m[:64, O_XQ:O_XQ + 1024] = \
            xb[:, qi * 16:(qi + 1) * 16, :].reshape(64, 1024)
        m[64, O_XQ:O_XQ + 1024] = 1.0
        xp = np.zeros((64, 66, 66), np.float32)
        xp[:, 1:65, 1:65] = xb
        flat = xp.reshape(64, 66 * 66)
        m[:64, O_X2:O_X2 + 4356] = flat
        m[64:, O_X2:O_X2 + 4355] = flat[:, 1:]
        # 18-row slab: padded rows qi*16 .. qi*16+18 (slab conv output row
        # rho = query row rho)
        slab = xp[:, qi * 16:qi * 16 + 18, :].reshape(64, 18 * 66)
        m[:64, O_XS2:O_XS2 + 1188] = slab
        m[64:, O_XS2:O_XS2 + 1187] = slab[:, 1:]
        in_maps.append({'allin': np.ascontiguousarray(m)})
    return in_maps


def kernel(**inputs):
    from concourse.bass_utils import run_bass_kernel_spmd
    if 'nc' not in _CACHE:
        _CACHE['nc'] = _build()
    nc = _CACHE['nc']
    in_maps = _prep_host(inputs)
    res = run_bass_kernel_spmd(nc, in_maps, core_ids=list(range(8)))
    out = np.zeros((B, C, H, W), np.float32)
    for c in range(8):
        b, qi = c // 4, c % 4
        out[b, :, qi * 16:(qi + 1) * 16, :] = \
            res.results[c]['out'].reshape(64, 16, 64)
    return out
